# revision 29
# baseline (speedup 1.0000x reference)
"""Trainium2 Bass kernel for the audio-visual attention model.

Math (per (b,t) sample, BT = 32*64 = 2048 of them):
    V   = video[b,t]                              # [48, 512]
    v   = relu(V @ W_video.T + b_video)           # [48, 512]
    a   = relu(audio[b,t] @ W_audio.T + b_audio)  # [512]
    inter   = a @ W_g.T                           # [48]
    content = v @ W_v.T + inter[:, None]          # [48, 48]
    z   = tanh(content) @ W_h.T                   # [48]
    alpha = softmax(z)
    out = alpha @ V                               # [512]

Strategy: data-parallel over BT across 8 cores (256 samples each, R = 256*48
= 12288 video rows per core).  The host pre-transposes the video shard to
V.T [512, 12288] (contraction dim on SBUF partitions), pre-arranges all
weights into their device layouts (so every weight DMA is contiguous per
partition), and converts the matmul chain to fp16 (values here are tiny, so
fp16's 10-bit mantissa gives ~7e-4 rel err while streaming the PE at
1 cycle/row -- float32 is 4x slower, float32r 2x).

Per-core device pipeline, over 8 superblocks of 1536 rows (3 sub-blocks of
512 rows each):
    vT.relu   = relu(W_video.T^T @ V.T + b_video)        PE + ACT  [h, rows]
    content.T = W_v.T^T @ vT.relu (+ ones^T @ inter)     PE        [48, rows]
    tanhc     = tanh(content.T)                          ACT
    zb        = (W_h.T replicated 128x)^T @ tanhc        PE        [128, rows]
                (z pre-broadcast to every partition -- no separate bcast)
    ezb       = exp(zb)   (no max-sub: |z| <= ~4)        ACT -> fp16 SBUF
    weighted  = V.T * ezb (in place, one 4-chunk op)     DVE 2x mode
    cT groups = halving-tree adds + small reduce         DVE (tree runs 2x)
    denom     = 48-group sums of ezb row 0               DVE
The audio phase (a.T, inter) shares PSUM pools with the main loop so no
pool-release barrier delays the first video DMA, and inter is flattened to a
row-major [1, 12288] single-partition row via two SBUF->SBUF DMAs on the
gpsimd ring (a DRAM roundtrip pays multi-microsecond HBM receipt latency
under load, and putting the DMA issue on the ACT ring entangles it with the
relu/tanh queue).  Dummy matmul bursts keep the PE clock gate warm through
the startup DMA fill.  Outputs are the unnormalized c.T [512, 256] (fp16)
plus denom [1, 256]; the host divides and transposes.
"""

import numpy as np

# Problem constants (hardcoded per harness contract).
B, T = 32, 64
ASIZE, VSIZE, HSIZE, MSIZE = 128, 512, 512, 48
NCORES = 8
BT = B * T                     # 2048
PER = BT // NCORES             # 256 samples per core
R = PER * MSIZE                # 12288 video rows per core
SUPER = 1536                   # rows per superblock (32 groups of 48)
NSB = R // SUPER               # 8 superblocks
SUB = 512                      # matmul moving-dim block
NSUB = SUPER // SUB            # 3
GPS = SUPER // MSIZE           # 32 groups per superblock

_cached = {}


def _build_nc():
    import concourse.bacc as bacc
    import concourse.mybir as mybir
    import concourse.tile as tile

    f32 = mybir.dt.float32
    f32r = mybir.dt.float32r
    f16 = mybir.dt.float16
    AF = mybir.ActivationFunctionType
    AX = mybir.AxisListType

    nc = bacc.Bacc(
        "TRN2",
        target_bir_lowering=False,
        debug=False,
        enable_asserts=False,
        num_devices=NCORES,
    )

    # ---- DRAM I/O ----
    vT_d = nc.dram_tensor("vT", [VSIZE, R], f16, kind="ExternalInput").ap()
    audioT_d = nc.dram_tensor("audioT", [ASIZE, PER], f16, kind="ExternalInput").ap()
    wvideoT_d = nc.dram_tensor("WvideoT", [128, VSIZE // 128, HSIZE], f16, kind="ExternalInput").ap()
    waudioT_d = nc.dram_tensor("WaudioT", [ASIZE, HSIZE], f16, kind="ExternalInput").ap()
    wgT_d = nc.dram_tensor("WgT", [128, HSIZE // 128, MSIZE], f16, kind="ExternalInput").ap()
    wvT_d = nc.dram_tensor("WvT", [128, HSIZE // 128, MSIZE], f16, kind="ExternalInput").ap()
    whT_d = nc.dram_tensor("WhT", [MSIZE, 1], f32, kind="ExternalInput").ap()
    bvideo_d = nc.dram_tensor("b_video", [128, HSIZE // 128], f32, kind="ExternalInput").ap()
    baudio_d = nc.dram_tensor("b_audio", [128, HSIZE // 128], f32, kind="ExternalInput").ap()
    cT_d = nc.dram_tensor("cT", [VSIZE, PER], f16, kind="ExternalOutput").ap()
    denom_d = nc.dram_tensor("denom", [1, PER], f16, kind="ExternalOutput").ap()

    KC = VSIZE // 128          # 4 contraction chunks for the main matmul
    HC = HSIZE // 128          # 4 h chunks

    with tile.TileContext(nc) as tc:
        with (
            tc.tile_pool(name="const", bufs=1) as const,
            tc.tile_pool(name="dram", bufs=1, space="DRAM") as dramp,
        ):
            # ---- constants / weights (host pre-arranged to device layout;
            # audio-path tensors first on the scalar ring: they gate the first
            # PE work) ----
            audioT_sb = const.tile([128, PER], f16)
            nc.scalar.dma_start(out=audioT_sb, in_=audioT_d)
            waudioT_sb = const.tile([128, HSIZE], f16)
            nc.scalar.dma_start(out=waudioT_sb, in_=waudioT_d)
            baudio_sb = const.tile([128, HC], f32)
            nc.scalar.dma_start(out=baudio_sb, in_=baudio_d)
            wgT_sb = const.tile([128, HC, MSIZE], f16)
            nc.scalar.dma_start(out=wgT_sb, in_=wgT_d)
            whT_sb = const.tile([MSIZE, 1], f32)
            nc.scalar.dma_start(out=whT_sb, in_=whT_d)
            bvideo_sb = const.tile([128, HC], f32)
            nc.scalar.dma_start(out=bvideo_sb, in_=bvideo_d)
            wvideoT_sb = const.tile([128, KC, HSIZE], f16)
            nc.scalar.dma_start(out=wvideoT_sb, in_=wvideoT_d)
            wvT_sb = const.tile([128, HC, MSIZE], f16)
            nc.scalar.dma_start(out=wvT_sb, in_=wvT_d)
            ones_m = const.tile([MSIZE, 128], f32)
            nc.vector.memset(ones_m, 1.0)
            whB_sb = const.tile([MSIZE, 128], f16)
            nc.scalar.mul(out=whB_sb, in_=ones_m, mul=whT_sb)
            # HAM warm-up: keep the PE busy during the initial DMA fill so the
            # clock gate is at 8/8 (2.4 GHz) before the real matmuls arrive
            warm_sb = const.tile([128, 64], f16)
            nc.vector.memset(warm_sb.bitcast(f32), 0.0)
            ones_f32 = const.tile([1, 128], f32)
            nc.vector.memset(ones_f32, 1.0)
            ones48 = const.tile([1, MSIZE], f16)
            nc.vector.tensor_copy(out=ones48, in_=ones_f32[:, :MSIZE])

            # persistent accumulators
            cT_acc = const.tile([128, KC, PER], f16)
            denom_sb = const.tile([1, PER], f16)
            interflat_all = const.tile([1, R], f16)

            # ---- pools (audio phase shares PSUM pools with the main loop so no
            # pool-release dependency delays the first video DMA) ----
            with (
                tc.tile_pool(name="vt", bufs=6) as vtp,
                tc.tile_pool(name="vrelu", bufs=4) as vrp,
                tc.tile_pool(name="tanhp", bufs=4) as thp,
                tc.tile_pool(name="small", bufs=3) as smp,
                tc.tile_pool(name="ezb", bufs=5) as ezp,
                tc.tile_pool(name="tree", bufs=2) as trp,
                tc.tile_pool(name="mm_ps", bufs=5, space="PSUM") as mm_ps,
                tc.tile_pool(name="ct_ps", bufs=2, space="PSUM") as ct_ps,
                tc.tile_pool(name="z_ps", bufs=1, space="PSUM") as z_ps,
            ):
                warm_ps = mm_ps.tile([64, 64], f32, tag="v_ps", name="warm_ps")

                def warm_burst(n):
                    for _ in range(n):
                        nc.tensor.matmul(
                            warm_ps, warm_sb[:, :64], warm_sb, start=True, stop=True
                        )

                warm_burst(100)

                # ---- audio phase: a.T = relu(W_audio.T^T @ audio.T + b_audio) ----
                aT_sb = const.tile([128, HC, PER], f16)
                for m in range(HC):
                    a_ps = mm_ps.tile([128, PER], f32, tag="v_ps", name=f"a_ps_{m}")
                    nc.tensor.matmul(
                        a_ps,
                        waudioT_sb[:, m * 128 : (m + 1) * 128],
                        audioT_sb,
                        start=True,
                        stop=True,
                    )
                    nc.scalar.activation(
                        out=aT_sb[:, m, :], in_=a_ps, func=AF.Relu,
                        bias=baudio_sb[:, m : m + 1],
                    )
                warm_burst(50)
                # inter[bt, m] = a @ W_g.T, natural layout for a flat DRAM write
                inter_sb = const.tile([128, PER // 128, MSIZE], f16)
                for t in range(PER // 128):
                    i_ps = ct_ps.tile([128, MSIZE], f32, tag="c_ps", name=f"i_ps_{t}")
                    for k in range(HC):
                        nc.tensor.matmul(
                            i_ps,
                            aT_sb[:, k, t * 128 : (t + 1) * 128],
                            wgT_sb[:, k, :],
                            start=(k == 0),
                            stop=(k == HC - 1),
                        )
                    nc.scalar.copy(out=inter_sb[:, t, :], in_=i_ps)
                # flatten inter [bt, m] row-major into a single-partition row
                # via SBUF->SBUF DMA (no HBM roundtrip; the video stream keeps
                # HBM receipt latency high)
                for t in range(PER // 128):
                    nc.gpsimd.dma_start(
                        out=interflat_all[:, t * 128 * MSIZE : (t + 1) * 128 * MSIZE],
                        in_=inter_sb[:, t, :],
                    )
                warm_burst(15)

                NGS = NSB * NSUB
                vt_t, vr_t, th_t, ez_t = {}, {}, {}, {}

                def emit_mains(gs):
                    sb, s = divmod(gs, NSUB)
                    if s == 0:
                        vt_t[sb] = vtp.tile([128, KC, SUPER], f16, tag="vt",
                                            name=f"vt_{sb}")
                        if sb == 0:
                            for ss in range(NSUB):
                                nc.sync.dma_start(
                                    out=vt_t[sb][:, :, ss * SUB : (ss + 1) * SUB],
                                    in_=vT_d[
                                        :, sb * SUPER + ss * SUB :
                                        sb * SUPER + (ss + 1) * SUB
                                    ].rearrange("(c p) n -> p c n", p=128),
                                )
                        else:
                            nc.sync.dma_start(
                                out=vt_t[sb],
                                in_=vT_d[:, sb * SUPER : (sb + 1) * SUPER].rearrange(
                                    "(c p) n -> p c n", p=128
                                ),
                            )
                        vr_t[sb] = vrp.tile([128, HC, SUPER], f16, tag="vrelu",
                                            name=f"vrelu_{sb}")
                        th_t[sb] = thp.tile([MSIZE, SUPER], f16, tag="tanhc",
                                            name=f"tanhc_{sb}")
                        ez_t[sb] = ezp.tile([128, SUPER], f16, tag="ezb",
                                            name=f"ezb_{sb}")
                    c0 = s * SUB
                    for m in range(HC):
                        v_ps = mm_ps.tile([128, SUB], f32, tag="v_ps",
                                          name=f"v_ps_{gs}_{m}")
                        for k in range(KC):
                            nc.tensor.matmul(
                                v_ps,
                                wvideoT_sb[:, k, m * 128 : (m + 1) * 128],
                                vt_t[sb][:, k, c0 : c0 + SUB],
                                start=(k == 0),
                                stop=(k == KC - 1),
                            )
                        nc.scalar.activation(
                            out=vr_t[sb][:, m, c0 : c0 + SUB], in_=v_ps,
                            func=AF.Relu, bias=bvideo_sb[:, m : m + 1],
                        )

                def emit_content(gs):
                    sb, s = divmod(gs, NSUB)
                    c0 = s * SUB
                    r0 = sb * SUPER
                    c_ps = ct_ps.tile([MSIZE, SUB], f32, tag="c_ps",
                                      name=f"c_ps_{gs}")
                    nc.tensor.matmul(
                        c_ps, ones48,
                        interflat_all[:, r0 + c0 : r0 + c0 + SUB],
                        start=True, stop=False,
                    )
                    for k in range(HC):
                        nc.tensor.matmul(
                            c_ps, wvT_sb[:, k, :], vr_t[sb][:, k, c0 : c0 + SUB],
                            start=False, stop=(k == HC - 1),
                        )
                    nc.scalar.activation(
                        out=th_t[sb][:, c0 : c0 + SUB], in_=c_ps, func=AF.Tanh
                    )

                def emit_score(gs):
                    sb, s = divmod(gs, NSUB)
                    c0 = s * SUB
                    zb_ps = z_ps.tile([128, SUB], f32, tag="zt_ps",
                                      name=f"zb_ps_{gs}")
                    nc.tensor.matmul(
                        zb_ps, whB_sb, th_t[sb][:, c0 : c0 + SUB],
                        start=True, stop=True,
                    )
                    nc.scalar.activation(
                        out=ez_t[sb][:, c0 : c0 + SUB], in_=zb_ps, func=AF.Exp
                    )
                    nc.vector.tensor_mul(
                        vt_t[sb][:, :, c0 : c0 + SUB],
                        vt_t[sb][:, :, c0 : c0 + SUB],
                        ez_t[sb][:, c0 : c0 + SUB]
                        .unsqueeze(1)
                        .broadcast_to([128, KC, SUB]),
                    )

                def emit_finalize(sb, ga=0, gb=GPS):
                    # finalize groups [ga, gb) of superblock sb (granular for
                    # the last superblock so the post-PE tail chain is short)
                    g0 = sb * GPS + ga
                    ng = gb - ga
                    lp = nc.allow_low_precision(
                        reason="fp16 group sums; fp32 internal accum"
                    )
                    lp.__enter__()
                    nc.vector.reduce_sum(
                        out=denom_sb[:, g0 : g0 + ng],
                        in_=ez_t[sb][0:1, ga * MSIZE : gb * MSIZE].rearrange(
                            "p (g n) -> p g n", n=MSIZE
                        ),
                        axis=AX.X,
                    )
                    tree = trp.tile([128, KC, ng, MSIZE // 2], f16, tag="tree",
                                    name=f"tree_{sb}_{ga}")
                    wv = vt_t[sb][:, :, ga * MSIZE : gb * MSIZE].rearrange(
                        "p c (g n) -> p c g n", n=MSIZE
                    )
                    nc.vector.tensor_add(
                        tree, wv[:, :, :, : MSIZE // 2], wv[:, :, :, MSIZE // 2 :]
                    )
                    nc.vector.tensor_add(
                        tree[:, :, :, : MSIZE // 4],
                        tree[:, :, :, : MSIZE // 4],
                        tree[:, :, :, MSIZE // 4 :],
                    )
                    nc.vector.tensor_add(
                        tree[:, :, :, : MSIZE // 8],
                        tree[:, :, :, : MSIZE // 8],
                        tree[:, :, :, MSIZE // 8 : MSIZE // 4],
                    )
                    nc.vector.reduce_sum(
                        out=cT_acc[:, :, g0 : g0 + ng],
                        in_=tree[:, :, :, : MSIZE // 8],
                        axis=AX.X,
                    )
                    lp.__exit__(None, None, None)
                    nc.sync.dma_start(
                        out=cT_d[:, g0 : g0 + ng].rearrange("(c p) n -> p c n", p=128),
                        in_=cT_acc[:, :, g0 : g0 + ng],
                    )

                # software-pipelined emission: mains(gs), content(gs-1),
                # score-chain(gs-2) -- every producer gets a full main block
                # to drain before its consumer issues
                # group-progress boundaries for granular last-superblock
                # finalize: after sub s the first (s+1)*SUB rows are weighted,
                # i.e. floor((s+1)*SUB/MSIZE) complete groups
                gcut = [min(GPS, (s + 1) * SUB // MSIZE) for s in range(NSUB)]
                for gs in range(NGS - 1):
                    emit_mains(gs)
                    if gs >= 1:
                        emit_content(gs - 1)
                    if gs >= 2:
                        emit_score(gs - 2)
                        if (gs - 2) % NSUB == NSUB - 1:
                            emit_finalize((gs - 2) // NSUB)
                # final iteration: collapse the pipeline lag step by step so
                # the last score/finalize chains overlap the last mains, and
                # only one small granule remains after the PE finishes
                gs = NGS - 1
                emit_content(gs - 1)
                emit_score(gs - 2)
                emit_mains(gs)
                emit_finalize(NSB - 1, 0, gcut[0])
                emit_content(gs)
                emit_score(gs - 1)
                emit_finalize(NSB - 1, gcut[0], gcut[1])
                emit_score(gs)
                emit_finalize(NSB - 1, gcut[1], GPS)
                nc.sync.dma_start(out=denom_d, in_=denom_sb)

    nc.compile()
    return nc


def _prep_in_maps(inputs):
    audio = np.ascontiguousarray(np.asarray(inputs["audio"], np.float32))
    video = np.ascontiguousarray(np.asarray(inputs["video"], np.float32))
    def dev_chunks(w):  # [C*128, X] -> [128, C, X] (partition-major chunks)
        a = np.asarray(w)
        return np.ascontiguousarray(a.reshape(-1, 128, a.shape[-1]).transpose(1, 0, 2))

    WvideoT = dev_chunks(np.asarray(inputs["W_video"], np.float32).T.astype(np.float16))
    WaudioT = np.ascontiguousarray(np.asarray(inputs["W_audio"], np.float32).T.astype(np.float16))
    WgT = dev_chunks(np.asarray(inputs["W_g"], np.float32).T.astype(np.float16))
    WvT = dev_chunks(np.asarray(inputs["W_v"], np.float32).T.astype(np.float16))
    WhT = np.ascontiguousarray(np.asarray(inputs["W_h"], np.float32).T)
    b_video = np.ascontiguousarray(
        np.asarray(inputs["b_video"], np.float32).reshape(-1, 128).T
    )
    b_audio = np.ascontiguousarray(
        np.asarray(inputs["b_audio"], np.float32).reshape(-1, 128).T
    )

    a2 = audio.reshape(BT, ASIZE).astype(np.float16)
    v2 = video.reshape(BT, MSIZE, VSIZE).astype(np.float16)
    in_maps = []
    for c in range(NCORES):
        sl = slice(c * PER, (c + 1) * PER)
        vT = np.ascontiguousarray(v2[sl].reshape(R, VSIZE).T)
        audioT = np.ascontiguousarray(a2[sl].T)
        in_maps.append(
            {
                "vT": vT,
                "audioT": audioT,
                "WvideoT": WvideoT,
                "WaudioT": WaudioT,
                "WgT": WgT,
                "WvT": WvT,
                "WhT": WhT,
                "b_video": b_video,
                "b_audio": b_audio,
            }
        )
    return in_maps


def _run(inputs, trace=False, **spmd_kwargs):
    from concourse.bass_utils import run_bass_kernel_spmd

    if "nc" not in _cached:
        _cached["nc"] = _build_nc()
    nc = _cached["nc"]
    in_maps = _prep_in_maps(inputs)
    res = run_bass_kernel_spmd(
        nc, in_maps, core_ids=list(range(NCORES)), trace=trace, **spmd_kwargs
    )
    parts = [(r["cT"].astype(np.float32) / r["denom"].astype(np.float32)).T for r in res.results]
    out = np.concatenate(parts, axis=0).reshape(B, T, VSIZE)
    return np.ascontiguousarray(out.astype(np.float32)), res


def kernel(**inputs):
    out, _ = _run(inputs, trace=False)
    return out



# revision 33
# speedup vs baseline: 1.0036x; 1.0036x over previous
"""Trainium2 Bass kernel for the audio-visual attention model.

Math (per (b,t) sample, BT = 32*64 = 2048 of them):
    V   = video[b,t]                              # [48, 512]
    v   = relu(V @ W_video.T + b_video)           # [48, 512]
    a   = relu(audio[b,t] @ W_audio.T + b_audio)  # [512]
    inter   = a @ W_g.T                           # [48]
    content = v @ W_v.T + inter[:, None]          # [48, 48]
    z   = tanh(content) @ W_h.T                   # [48]
    alpha = softmax(z)
    out = alpha @ V                               # [512]

Strategy: data-parallel over BT across 8 cores (256 samples each, R = 256*48
= 12288 video rows per core).  The host pre-transposes the video shard to
V.T [512, 12288] (contraction dim on SBUF partitions), pre-arranges all
weights into their device layouts (so every weight DMA is contiguous per
partition), and converts the matmul chain to fp16 (values here are tiny, so
fp16's 10-bit mantissa gives ~7e-4 rel err while streaming the PE at
1 cycle/row -- float32 is 4x slower, float32r 2x).

Per-core device pipeline, over 8 superblocks of 1536 rows (3 sub-blocks of
512 rows each):
    vT.relu   = relu(W_video.T^T @ V.T + b_video)        PE + ACT  [h, rows]
    content.T = W_v.T^T @ vT.relu (+ ones^T @ inter)     PE        [48, rows]
    tanhc     = tanh(content.T)                          ACT
    zb        = (W_h.T replicated 128x)^T @ tanhc        PE        [128, rows]
                (z pre-broadcast to every partition -- no separate bcast)
    ezb       = exp(zb)   (no max-sub: |z| <= ~4)        ACT -> fp16 SBUF
    weighted  = V.T * ezb (in place, one 4-chunk op)     DVE 2x mode
    cT groups = halving-tree adds + small reduce         DVE (tree runs 2x)
    denom     = 48-group sums of ezb row 0               DVE
The audio phase (a.T, inter) shares PSUM pools with the main loop so no
pool-release barrier delays the first video DMA, and inter is flattened to a
row-major [1, 12288] single-partition row via two SBUF->SBUF DMAs on the
gpsimd ring (a DRAM roundtrip pays multi-microsecond HBM receipt latency
under load, and putting the DMA issue on the ACT ring entangles it with the
relu/tanh queue).  Dummy matmul bursts keep the PE clock gate warm through
the startup DMA fill.  Outputs are the unnormalized c.T [512, 256] (fp16)
plus denom [1, 256]; the host divides and transposes.
"""

import numpy as np

# Problem constants (hardcoded per harness contract).
B, T = 32, 64
ASIZE, VSIZE, HSIZE, MSIZE = 128, 512, 512, 48
NCORES = 8
BT = B * T                     # 2048
PER = BT // NCORES             # 256 samples per core
R = PER * MSIZE                # 12288 video rows per core
SUPER = 1536                   # rows per superblock (32 groups of 48)
NSB = R // SUPER               # 8 superblocks
SUB = 512                      # matmul moving-dim block
NSUB = SUPER // SUB            # 3
GPS = SUPER // MSIZE           # 32 groups per superblock

_cached = {}


def _build_nc():
    import concourse.bacc as bacc
    import concourse.mybir as mybir
    import concourse.tile as tile

    f32 = mybir.dt.float32
    f32r = mybir.dt.float32r
    f16 = mybir.dt.float16
    AF = mybir.ActivationFunctionType
    AX = mybir.AxisListType

    nc = bacc.Bacc(
        "TRN2",
        target_bir_lowering=False,
        debug=False,
        enable_asserts=False,
        num_devices=NCORES,
    )

    # ---- DRAM I/O ----
    vT_d = nc.dram_tensor("vT", [VSIZE, R], f16, kind="ExternalInput").ap()
    audioT_d = nc.dram_tensor("audioT", [ASIZE, PER], f16, kind="ExternalInput").ap()
    wvideoT_d = nc.dram_tensor("WvideoT", [128, VSIZE // 128, HSIZE], f16, kind="ExternalInput").ap()
    waudioT_d = nc.dram_tensor("WaudioT", [ASIZE, HSIZE], f16, kind="ExternalInput").ap()
    wgT_d = nc.dram_tensor("WgT", [128, HSIZE // 128, MSIZE], f16, kind="ExternalInput").ap()
    wvT_d = nc.dram_tensor("WvT", [128, HSIZE // 128, MSIZE], f16, kind="ExternalInput").ap()
    whT_d = nc.dram_tensor("WhT", [MSIZE, 1], f32, kind="ExternalInput").ap()
    bvideo_d = nc.dram_tensor("b_video", [128, HSIZE // 128], f32, kind="ExternalInput").ap()
    baudio_d = nc.dram_tensor("b_audio", [128, HSIZE // 128], f32, kind="ExternalInput").ap()
    cT_d = nc.dram_tensor("cT", [VSIZE, PER], f16, kind="ExternalOutput").ap()
    denom_d = nc.dram_tensor("denom", [1, PER], f16, kind="ExternalOutput").ap()

    KC = VSIZE // 128          # 4 contraction chunks for the main matmul
    HC = HSIZE // 128          # 4 h chunks

    with tile.TileContext(nc) as tc:
        with (
            tc.tile_pool(name="const", bufs=1) as const,
            tc.tile_pool(name="dram", bufs=1, space="DRAM") as dramp,
        ):
            # ---- constants / weights (host pre-arranged to device layout;
            # audio-path tensors first on the scalar ring: they gate the first
            # PE work) ----
            audioT_sb = const.tile([128, PER], f16)
            nc.scalar.dma_start(out=audioT_sb, in_=audioT_d)
            waudioT_sb = const.tile([128, HSIZE], f16)
            nc.scalar.dma_start(out=waudioT_sb, in_=waudioT_d)
            baudio_sb = const.tile([128, HC], f32)
            nc.scalar.dma_start(out=baudio_sb, in_=baudio_d)
            wgT_sb = const.tile([128, HC, MSIZE], f16)
            nc.scalar.dma_start(out=wgT_sb, in_=wgT_d)
            whT_sb = const.tile([MSIZE, 1], f32)
            nc.scalar.dma_start(out=whT_sb, in_=whT_d)
            bvideo_sb = const.tile([128, HC], f32)
            nc.scalar.dma_start(out=bvideo_sb, in_=bvideo_d)
            wvideoT_sb = const.tile([128, KC, HSIZE], f16)
            nc.scalar.dma_start(out=wvideoT_sb, in_=wvideoT_d)
            wvT_sb = const.tile([128, HC, MSIZE], f16)
            nc.scalar.dma_start(out=wvT_sb, in_=wvT_d)
            ones_m = const.tile([MSIZE, 128], f32)
            nc.vector.memset(ones_m, 1.0)
            whB_sb = const.tile([MSIZE, 128], f16)
            nc.scalar.mul(out=whB_sb, in_=ones_m, mul=whT_sb)
            # HAM warm-up: keep the PE busy during the initial DMA fill so the
            # clock gate is at 8/8 (2.4 GHz) before the real matmuls arrive
            warm_sb = const.tile([128, 64], f16)
            nc.vector.memset(warm_sb.bitcast(f32), 0.0)
            ones_f32 = const.tile([1, 128], f32)
            nc.vector.memset(ones_f32, 1.0)
            ones48 = const.tile([1, MSIZE], f16)
            nc.vector.tensor_copy(out=ones48, in_=ones_f32[:, :MSIZE])

            # persistent accumulators
            cT_acc = const.tile([128, KC, PER], f16)
            denom_sb = const.tile([1, PER], f16)
            interflat_all = const.tile([1, R], f16)

            # ---- pools (audio phase shares PSUM pools with the main loop so no
            # pool-release dependency delays the first video DMA) ----
            with (
                tc.tile_pool(name="vt", bufs=6) as vtp,
                tc.tile_pool(name="vrelu", bufs=4) as vrp,
                tc.tile_pool(name="tanhp", bufs=4) as thp,
                tc.tile_pool(name="small", bufs=3) as smp,
                tc.tile_pool(name="ezb", bufs=5) as ezp,
                tc.tile_pool(name="tree", bufs=2) as trp,
                tc.tile_pool(name="mm_ps", bufs=4, space="PSUM") as mm_ps,
                tc.tile_pool(name="ct_ps", bufs=2, space="PSUM") as ct_ps,
                tc.tile_pool(name="z_ps", bufs=2, space="PSUM") as z_ps,
            ):
                warm_ps = mm_ps.tile([64, 64], f32, tag="v_ps", name="warm_ps")

                def warm_burst(n):
                    for _ in range(n):
                        nc.tensor.matmul(
                            warm_ps, warm_sb[:, :64], warm_sb, start=True, stop=True
                        )

                warm_burst(100)

                # ---- audio phase: a.T = relu(W_audio.T^T @ audio.T + b_audio) ----
                aT_sb = const.tile([128, HC, PER], f16)
                for m in range(HC):
                    a_ps = mm_ps.tile([128, PER], f32, tag="v_ps", name=f"a_ps_{m}")
                    nc.tensor.matmul(
                        a_ps,
                        waudioT_sb[:, m * 128 : (m + 1) * 128],
                        audioT_sb,
                        start=True,
                        stop=True,
                    )
                    nc.scalar.activation(
                        out=aT_sb[:, m, :], in_=a_ps, func=AF.Relu,
                        bias=baudio_sb[:, m : m + 1],
                    )
                warm_burst(50)
                # inter[bt, m] = a @ W_g.T, natural layout for a flat DRAM write
                inter_sb = const.tile([128, PER // 128, MSIZE], f16)
                for t in range(PER // 128):
                    i_ps = ct_ps.tile([128, MSIZE], f32, tag="c_ps", name=f"i_ps_{t}")
                    for k in range(HC):
                        nc.tensor.matmul(
                            i_ps,
                            aT_sb[:, k, t * 128 : (t + 1) * 128],
                            wgT_sb[:, k, :],
                            start=(k == 0),
                            stop=(k == HC - 1),
                        )
                    nc.scalar.copy(out=inter_sb[:, t, :], in_=i_ps)
                # flatten inter [bt, m] row-major into a single-partition row
                # via SBUF->SBUF DMA (no HBM roundtrip; the video stream keeps
                # HBM receipt latency high)
                for t in range(PER // 128):
                    nc.gpsimd.dma_start(
                        out=interflat_all[:, t * 128 * MSIZE : (t + 1) * 128 * MSIZE],
                        in_=inter_sb[:, t, :],
                    )
                warm_burst(50)

                NGS = NSB * NSUB
                vt_t, vr_t, th_t, ez_t = {}, {}, {}, {}

                def emit_mains(gs):
                    sb, s = divmod(gs, NSUB)
                    if s == 0:
                        vt_t[sb] = vtp.tile([128, KC, SUPER], f16, tag="vt",
                                            name=f"vt_{sb}")
                        if sb == 0:
                            for ss in range(NSUB):
                                nc.sync.dma_start(
                                    out=vt_t[sb][:, :, ss * SUB : (ss + 1) * SUB],
                                    in_=vT_d[
                                        :, sb * SUPER + ss * SUB :
                                        sb * SUPER + (ss + 1) * SUB
                                    ].rearrange("(c p) n -> p c n", p=128),
                                )
                        else:
                            nc.sync.dma_start(
                                out=vt_t[sb],
                                in_=vT_d[:, sb * SUPER : (sb + 1) * SUPER].rearrange(
                                    "(c p) n -> p c n", p=128
                                ),
                            )
                        vr_t[sb] = vrp.tile([128, HC, SUPER], f16, tag="vrelu",
                                            name=f"vrelu_{sb}")
                        th_t[sb] = thp.tile([MSIZE, SUPER], f16, tag="tanhc",
                                            name=f"tanhc_{sb}")
                        ez_t[sb] = ezp.tile([128, SUPER], f16, tag="ezb",
                                            name=f"ezb_{sb}")
                    c0 = s * SUB
                    for m in range(HC):
                        v_ps = mm_ps.tile([128, SUB], f32, tag="v_ps",
                                          name=f"v_ps_{gs}_{m}")
                        for k in range(KC):
                            nc.tensor.matmul(
                                v_ps,
                                wvideoT_sb[:, k, m * 128 : (m + 1) * 128],
                                vt_t[sb][:, k, c0 : c0 + SUB],
                                start=(k == 0),
                                stop=(k == KC - 1),
                            )
                        nc.scalar.activation(
                            out=vr_t[sb][:, m, c0 : c0 + SUB], in_=v_ps,
                            func=AF.Relu, bias=bvideo_sb[:, m : m + 1],
                        )

                def emit_content(gs):
                    sb, s = divmod(gs, NSUB)
                    c0 = s * SUB
                    r0 = sb * SUPER
                    c_ps = ct_ps.tile([MSIZE, SUB], f32, tag="c_ps",
                                      name=f"c_ps_{gs}")
                    nc.tensor.matmul(
                        c_ps, ones48,
                        interflat_all[:, r0 + c0 : r0 + c0 + SUB],
                        start=True, stop=False,
                    )
                    for k in range(HC):
                        nc.tensor.matmul(
                            c_ps, wvT_sb[:, k, :], vr_t[sb][:, k, c0 : c0 + SUB],
                            start=False, stop=(k == HC - 1),
                        )
                    nc.scalar.activation(
                        out=th_t[sb][:, c0 : c0 + SUB], in_=c_ps, func=AF.Tanh
                    )

                def emit_score(gs):
                    sb, s = divmod(gs, NSUB)
                    c0 = s * SUB
                    zb_ps = z_ps.tile([128, SUB], f32, tag="zt_ps",
                                      name=f"zb_ps_{gs}")
                    nc.tensor.matmul(
                        zb_ps, whB_sb, th_t[sb][:, c0 : c0 + SUB],
                        start=True, stop=True,
                    )
                    nc.scalar.activation(
                        out=ez_t[sb][:, c0 : c0 + SUB], in_=zb_ps, func=AF.Exp
                    )
                    nc.vector.tensor_mul(
                        vt_t[sb][:, :, c0 : c0 + SUB],
                        vt_t[sb][:, :, c0 : c0 + SUB],
                        ez_t[sb][:, c0 : c0 + SUB]
                        .unsqueeze(1)
                        .broadcast_to([128, KC, SUB]),
                    )

                def emit_finalize(sb, ga=0, gb=GPS):
                    # finalize groups [ga, gb) of superblock sb (granular for
                    # the last superblock so the post-PE tail chain is short)
                    g0 = sb * GPS + ga
                    ng = gb - ga
                    lp = nc.allow_low_precision(
                        reason="fp16 group sums; fp32 internal accum"
                    )
                    lp.__enter__()
                    nc.vector.reduce_sum(
                        out=denom_sb[:, g0 : g0 + ng],
                        in_=ez_t[sb][0:1, ga * MSIZE : gb * MSIZE].rearrange(
                            "p (g n) -> p g n", n=MSIZE
                        ),
                        axis=AX.X,
                    )
                    tree = trp.tile([128, KC, ng, MSIZE // 2], f16, tag="tree",
                                    name=f"tree_{sb}_{ga}")
                    wv = vt_t[sb][:, :, ga * MSIZE : gb * MSIZE].rearrange(
                        "p c (g n) -> p c g n", n=MSIZE
                    )
                    nc.vector.tensor_add(
                        tree, wv[:, :, :, : MSIZE // 2], wv[:, :, :, MSIZE // 2 :]
                    )
                    nc.vector.tensor_add(
                        tree[:, :, :, : MSIZE // 4],
                        tree[:, :, :, : MSIZE // 4],
                        tree[:, :, :, MSIZE // 4 :],
                    )
                    nc.vector.tensor_add(
                        tree[:, :, :, : MSIZE // 8],
                        tree[:, :, :, : MSIZE // 8],
                        tree[:, :, :, MSIZE // 8 : MSIZE // 4],
                    )
                    nc.vector.reduce_sum(
                        out=cT_acc[:, :, g0 : g0 + ng],
                        in_=tree[:, :, :, : MSIZE // 8],
                        axis=AX.X,
                    )
                    lp.__exit__(None, None, None)
                    nc.sync.dma_start(
                        out=cT_d[:, g0 : g0 + ng].rearrange("(c p) n -> p c n", p=128),
                        in_=cT_acc[:, :, g0 : g0 + ng],
                    )

                # software-pipelined emission: mains(gs), content(gs-1),
                # score-chain(gs-2) -- every producer gets a full main block
                # to drain before its consumer issues
                # group-progress boundaries for granular last-superblock
                # finalize: after sub s the first (s+1)*SUB rows are weighted,
                # i.e. floor((s+1)*SUB/MSIZE) complete groups
                gcut = [min(GPS, (s + 1) * SUB // MSIZE) for s in range(NSUB)]
                for gs in range(NGS - 1):
                    emit_mains(gs)
                    if gs >= 1:
                        emit_content(gs - 1)
                    if gs >= 2:
                        emit_score(gs - 2)
                        if (gs - 2) % NSUB == NSUB - 1:
                            emit_finalize((gs - 2) // NSUB)
                # final iteration: collapse the pipeline lag step by step so
                # the last score/finalize chains overlap the last mains, and
                # only one small granule remains after the PE finishes
                gs = NGS - 1
                emit_content(gs - 1)
                emit_score(gs - 2)
                emit_mains(gs)
                emit_finalize(NSB - 1, 0, gcut[0])
                emit_content(gs)
                emit_score(gs - 1)
                emit_finalize(NSB - 1, gcut[0], gcut[1])
                emit_score(gs)
                emit_finalize(NSB - 1, gcut[1], GPS)
                nc.sync.dma_start(out=denom_d, in_=denom_sb)

    nc.compile()
    return nc


def _prep_in_maps(inputs):
    audio = np.ascontiguousarray(np.asarray(inputs["audio"], np.float32))
    video = np.ascontiguousarray(np.asarray(inputs["video"], np.float32))
    def dev_chunks(w):  # [C*128, X] -> [128, C, X] (partition-major chunks)
        a = np.asarray(w)
        return np.ascontiguousarray(a.reshape(-1, 128, a.shape[-1]).transpose(1, 0, 2))

    WvideoT = dev_chunks(np.asarray(inputs["W_video"], np.float32).T.astype(np.float16))
    WaudioT = np.ascontiguousarray(np.asarray(inputs["W_audio"], np.float32).T.astype(np.float16))
    WgT = dev_chunks(np.asarray(inputs["W_g"], np.float32).T.astype(np.float16))
    WvT = dev_chunks(np.asarray(inputs["W_v"], np.float32).T.astype(np.float16))
    WhT = np.ascontiguousarray(np.asarray(inputs["W_h"], np.float32).T)
    b_video = np.ascontiguousarray(
        np.asarray(inputs["b_video"], np.float32).reshape(-1, 128).T
    )
    b_audio = np.ascontiguousarray(
        np.asarray(inputs["b_audio"], np.float32).reshape(-1, 128).T
    )

    a2 = audio.reshape(BT, ASIZE).astype(np.float16)
    v2 = video.reshape(BT, MSIZE, VSIZE).astype(np.float16)
    in_maps = []
    for c in range(NCORES):
        sl = slice(c * PER, (c + 1) * PER)
        vT = np.ascontiguousarray(v2[sl].reshape(R, VSIZE).T)
        audioT = np.ascontiguousarray(a2[sl].T)
        in_maps.append(
            {
                "vT": vT,
                "audioT": audioT,
                "WvideoT": WvideoT,
                "WaudioT": WaudioT,
                "WgT": WgT,
                "WvT": WvT,
                "WhT": WhT,
                "b_video": b_video,
                "b_audio": b_audio,
            }
        )
    return in_maps


def _run(inputs, trace=False, **spmd_kwargs):
    from concourse.bass_utils import run_bass_kernel_spmd

    if "nc" not in _cached:
        _cached["nc"] = _build_nc()
    nc = _cached["nc"]
    in_maps = _prep_in_maps(inputs)
    res = run_bass_kernel_spmd(
        nc, in_maps, core_ids=list(range(NCORES)), trace=trace, **spmd_kwargs
    )
    parts = [(r["cT"].astype(np.float32) / r["denom"].astype(np.float32)).T for r in res.results]
    out = np.concatenate(parts, axis=0).reshape(B, T, VSIZE)
    return np.ascontiguousarray(out.astype(np.float32)), res


def kernel(**inputs):
    out, _ = _run(inputs, trace=False)
    return out



# revision 37
# speedup vs baseline: 1.0107x; 1.0071x over previous
"""Trainium2 Bass kernel for the audio-visual attention model.

Math (per (b,t) sample, BT = 32*64 = 2048 of them):
    V   = video[b,t]                              # [48, 512]
    v   = relu(V @ W_video.T + b_video)           # [48, 512]
    a   = relu(audio[b,t] @ W_audio.T + b_audio)  # [512]
    inter   = a @ W_g.T                           # [48]
    content = v @ W_v.T + inter[:, None]          # [48, 48]
    z   = tanh(content) @ W_h.T                   # [48]
    alpha = softmax(z)
    out = alpha @ V                               # [512]

Strategy: data-parallel over BT across 8 cores (256 samples each, R = 256*48
= 12288 video rows per core).  The host pre-transposes the video shard to
V.T [512, 12288] (contraction dim on SBUF partitions), pre-arranges all
weights into their device layouts (so every weight DMA is contiguous per
partition), and converts the matmul chain to fp16 (values here are tiny, so
fp16's 10-bit mantissa gives ~7e-4 rel err while streaming the PE at
1 cycle/row -- float32 is 4x slower, float32r 2x).

Per-core device pipeline, over 8 superblocks of 1536 rows (3 sub-blocks of
512 rows each):
    vT.relu   = relu(W_video.T^T @ V.T + b_video)        PE + ACT  [h, rows]
    content.T = W_v.T^T @ vT.relu (+ ones^T @ inter)     PE        [48, rows]
    tanhc     = tanh(content.T)                          ACT
    zb        = (W_h.T replicated 128x)^T @ tanhc        PE        [128, rows]
                (z pre-broadcast to every partition -- no separate bcast)
    ezb       = exp(zb)   (no max-sub: |z| <= ~4)        ACT -> fp16 SBUF
    weighted  = V.T * ezb (in place, one 4-chunk op)     DVE 2x mode
    cT groups = halving-tree adds + small reduce         DVE (tree runs 2x)
    denom     = 48-group sums of ezb row 0               DVE
The audio phase (a.T, inter) shares PSUM pools with the main loop so no
pool-release barrier delays the first video DMA, and inter is flattened to a
row-major [1, 12288] single-partition row via two SBUF->SBUF DMAs on the
gpsimd ring (a DRAM roundtrip pays multi-microsecond HBM receipt latency
under load, and putting the DMA issue on the ACT ring entangles it with the
relu/tanh queue).  Dummy matmul bursts keep the PE clock gate warm through
the startup DMA fill.  Outputs are the unnormalized c.T [512, 256] (fp16)
plus denom [1, 256]; the host divides and transposes.

Tail handling: the cT/denom output DMAs issue on the sync ring (the gpsimd
ring would head-of-line block behind their DVE-reduce waits), the last
superblock's finalize is emitted in 48-row-group granules interleaved with
the final score chain, and the closing iteration collapses the
content/score pipeline lag step by step -- together this cuts the post-PE
serial tail from ~19us to ~6us.  (Measured: the fp16 PE stream is the
roofline here -- 512-col matmuls sustain 259ns wall-to-wall regardless of
ldweights elision; fp8e4 DoubleRow measures 2.37x fp16 throughput, not the
cost model's 4x, so the 3-pass residual-compensated fp8 scheme that fits
the 2e-2 error gate is a net PE loss and was reverted.)
"""

import numpy as np

# Problem constants (hardcoded per harness contract).
B, T = 32, 64
ASIZE, VSIZE, HSIZE, MSIZE = 128, 512, 512, 48
NCORES = 8
BT = B * T                     # 2048
PER = BT // NCORES             # 256 samples per core
R = PER * MSIZE                # 12288 video rows per core
SUPER = 1536                   # rows per superblock (32 groups of 48)
NSB = R // SUPER               # 8 superblocks
SUB = 512                      # matmul moving-dim block
NSUB = SUPER // SUB            # 3
GPS = SUPER // MSIZE           # 32 groups per superblock

_cached = {}


def _build_nc():
    import concourse.bacc as bacc
    import concourse.mybir as mybir
    import concourse.tile as tile

    f32 = mybir.dt.float32
    f32r = mybir.dt.float32r
    f16 = mybir.dt.float16
    AF = mybir.ActivationFunctionType
    AX = mybir.AxisListType

    nc = bacc.Bacc(
        "TRN2",
        target_bir_lowering=False,
        debug=False,
        enable_asserts=False,
        num_devices=NCORES,
    )

    # ---- DRAM I/O ----
    vT_d = nc.dram_tensor("vT", [VSIZE, R], f16, kind="ExternalInput").ap()
    audioT_d = nc.dram_tensor("audioT", [ASIZE, PER], f16, kind="ExternalInput").ap()
    wvideoT_d = nc.dram_tensor("WvideoT", [128, VSIZE // 128, HSIZE], f16, kind="ExternalInput").ap()
    waudioT_d = nc.dram_tensor("WaudioT", [ASIZE, HSIZE], f16, kind="ExternalInput").ap()
    wgT_d = nc.dram_tensor("WgT", [128, HSIZE // 128, MSIZE], f16, kind="ExternalInput").ap()
    wvT_d = nc.dram_tensor("WvT", [128, HSIZE // 128, MSIZE], f16, kind="ExternalInput").ap()
    whT_d = nc.dram_tensor("WhT", [MSIZE, 1], f32, kind="ExternalInput").ap()
    bvideo_d = nc.dram_tensor("b_video", [128, HSIZE // 128], f32, kind="ExternalInput").ap()
    baudio_d = nc.dram_tensor("b_audio", [128, HSIZE // 128], f32, kind="ExternalInput").ap()
    cT_d = nc.dram_tensor("cT", [VSIZE, PER], f16, kind="ExternalOutput").ap()
    denom_d = nc.dram_tensor("denom", [1, PER], f16, kind="ExternalOutput").ap()

    KC = VSIZE // 128          # 4 contraction chunks for the main matmul
    HC = HSIZE // 128          # 4 h chunks

    with tile.TileContext(nc) as tc:
        with (
            tc.tile_pool(name="const", bufs=1) as const,
            tc.tile_pool(name="dram", bufs=1, space="DRAM") as dramp,
        ):
            # ---- constants / weights (host pre-arranged to device layout;
            # audio-path tensors first on the scalar ring: they gate the first
            # PE work) ----
            wvideoT_sb = const.tile([128, KC, HSIZE], f16)
            nc.scalar.dma_start(out=wvideoT_sb, in_=wvideoT_d)
            bvideo_sb = const.tile([128, HC], f32)
            nc.scalar.dma_start(out=bvideo_sb, in_=bvideo_d)
            audioT_sb = const.tile([128, PER], f16)
            nc.scalar.dma_start(out=audioT_sb, in_=audioT_d)
            waudioT_sb = const.tile([128, HSIZE], f16)
            nc.scalar.dma_start(out=waudioT_sb, in_=waudioT_d)
            baudio_sb = const.tile([128, HC], f32)
            nc.scalar.dma_start(out=baudio_sb, in_=baudio_d)
            wgT_sb = const.tile([128, HC, MSIZE], f16)
            nc.scalar.dma_start(out=wgT_sb, in_=wgT_d)
            whT_sb = const.tile([MSIZE, 1], f32)
            nc.scalar.dma_start(out=whT_sb, in_=whT_d)
            wvT_sb = const.tile([128, HC, MSIZE], f16)
            nc.scalar.dma_start(out=wvT_sb, in_=wvT_d)
            ones_m = const.tile([MSIZE, 128], f32)
            nc.vector.memset(ones_m, 1.0)
            whB_sb = const.tile([MSIZE, 128], f16)
            nc.scalar.mul(out=whB_sb, in_=ones_m, mul=whT_sb)
            # HAM warm-up: keep the PE busy during the initial DMA fill so the
            # clock gate is at 8/8 (2.4 GHz) before the real matmuls arrive
            warm_sb = const.tile([128, 64], f16)
            nc.vector.memset(warm_sb.bitcast(f32), 0.0)
            ones_f32 = const.tile([1, 128], f32)
            nc.vector.memset(ones_f32, 1.0)
            ones48 = const.tile([1, MSIZE], f16)
            nc.vector.tensor_copy(out=ones48, in_=ones_f32[:, :MSIZE])

            # persistent accumulators
            cT_acc = const.tile([128, KC, PER], f16)
            denom_sb = const.tile([1, PER], f16)
            interflat_all = const.tile([1, R], f16)

            # ---- pools (audio phase shares PSUM pools with the main loop so no
            # pool-release dependency delays the first video DMA) ----
            with (
                tc.tile_pool(name="vt", bufs=6) as vtp,
                tc.tile_pool(name="vrelu", bufs=4) as vrp,
                tc.tile_pool(name="tanhp", bufs=4) as thp,
                tc.tile_pool(name="small", bufs=3) as smp,
                tc.tile_pool(name="ezb", bufs=5) as ezp,
                tc.tile_pool(name="tree", bufs=2) as trp,
                tc.tile_pool(name="mm_ps", bufs=4, space="PSUM") as mm_ps,
                tc.tile_pool(name="ct_ps", bufs=2, space="PSUM") as ct_ps,
                tc.tile_pool(name="z_ps", bufs=2, space="PSUM") as z_ps,
            ):
                warm_ps = mm_ps.tile([64, 64], f32, tag="v_ps", name="warm_ps")

                def warm_burst(n):
                    for _ in range(n):
                        nc.tensor.matmul(
                            warm_ps, warm_sb[:, :64], warm_sb, start=True, stop=True
                        )

                warm_burst(100)

                def emit_audio():
                    # audio phase: a.T = relu(W_audio.T^T @ audio.T + b_audio)
                    # -- emitted after mains(0) so the heavy video stream
                    # starts as soon as its DMAs land.  PSUM comes from the
                    # ct/z pools (idle until gs>=1) so the audio matmuls
                    # don't contend with mains(0)'s v_ps tiles.
                    aT_sb = const.tile([128, HC, PER], f16)
                    for m in range(HC):
                        a_ps = z_ps.tile([128, PER], f32, tag="zt_ps",
                                         name=f"a_ps_{m}")
                        nc.tensor.matmul(
                            a_ps,
                            waudioT_sb[:, m * 128 : (m + 1) * 128],
                            audioT_sb,
                            start=True,
                            stop=True,
                        )
                        nc.scalar.activation(
                            out=aT_sb[:, m, :], in_=a_ps, func=AF.Relu,
                            bias=baudio_sb[:, m : m + 1],
                        )
                    # inter[bt, m] = a @ W_g.T, natural layout for a flat write
                    inter_sb = const.tile([128, PER // 128, MSIZE], f16)
                    for t in range(PER // 128):
                        i_ps = ct_ps.tile([128, MSIZE], f32, tag="c_ps",
                                          name=f"i_ps_{t}")
                        for k in range(HC):
                            nc.tensor.matmul(
                                i_ps,
                                aT_sb[:, k, t * 128 : (t + 1) * 128],
                                wgT_sb[:, k, :],
                                start=(k == 0),
                                stop=(k == HC - 1),
                            )
                        nc.scalar.copy(out=inter_sb[:, t, :], in_=i_ps)
                    # flatten inter [bt, m] row-major into a single-partition
                    # row via SBUF->SBUF DMA (no HBM roundtrip)
                    for t in range(PER // 128):
                        nc.gpsimd.dma_start(
                            out=interflat_all[:, t * 128 * MSIZE : (t + 1) * 128 * MSIZE],
                            in_=inter_sb[:, t, :],
                        )

                NGS = NSB * NSUB
                vt_t, vr_t, th_t, ez_t = {}, {}, {}, {}

                def emit_mains(gs):
                    sb, s = divmod(gs, NSUB)
                    if s == 0:
                        vt_t[sb] = vtp.tile([128, KC, SUPER], f16, tag="vt",
                                            name=f"vt_{sb}")
                        if sb == 0:
                            for ss in range(NSUB):
                                nc.sync.dma_start(
                                    out=vt_t[sb][:, :, ss * SUB : (ss + 1) * SUB],
                                    in_=vT_d[
                                        :, sb * SUPER + ss * SUB :
                                        sb * SUPER + (ss + 1) * SUB
                                    ].rearrange("(c p) n -> p c n", p=128),
                                )
                        else:
                            nc.sync.dma_start(
                                out=vt_t[sb],
                                in_=vT_d[:, sb * SUPER : (sb + 1) * SUPER].rearrange(
                                    "(c p) n -> p c n", p=128
                                ),
                            )
                        vr_t[sb] = vrp.tile([128, HC, SUPER], f16, tag="vrelu",
                                            name=f"vrelu_{sb}")
                        th_t[sb] = thp.tile([MSIZE, SUPER], f16, tag="tanhc",
                                            name=f"tanhc_{sb}")
                        ez_t[sb] = ezp.tile([128, SUPER], f16, tag="ezb",
                                            name=f"ezb_{sb}")
                    c0 = s * SUB
                    for m in range(HC):
                        v_ps = mm_ps.tile([128, SUB], f32, tag="v_ps",
                                          name=f"v_ps_{gs}_{m}")
                        for k in range(KC):
                            nc.tensor.matmul(
                                v_ps,
                                wvideoT_sb[:, k, m * 128 : (m + 1) * 128],
                                vt_t[sb][:, k, c0 : c0 + SUB],
                                start=(k == 0),
                                stop=(k == KC - 1),
                            )
                        nc.scalar.activation(
                            out=vr_t[sb][:, m, c0 : c0 + SUB], in_=v_ps,
                            func=AF.Relu, bias=bvideo_sb[:, m : m + 1],
                        )

                def emit_content(gs):
                    sb, s = divmod(gs, NSUB)
                    c0 = s * SUB
                    r0 = sb * SUPER
                    c_ps = ct_ps.tile([MSIZE, SUB], f32, tag="c_ps",
                                      name=f"c_ps_{gs}")
                    nc.tensor.matmul(
                        c_ps, ones48,
                        interflat_all[:, r0 + c0 : r0 + c0 + SUB],
                        start=True, stop=False,
                    )
                    for k in range(HC):
                        nc.tensor.matmul(
                            c_ps, wvT_sb[:, k, :], vr_t[sb][:, k, c0 : c0 + SUB],
                            start=False, stop=(k == HC - 1),
                        )
                    nc.scalar.activation(
                        out=th_t[sb][:, c0 : c0 + SUB], in_=c_ps, func=AF.Tanh
                    )

                def emit_score(gs):
                    sb, s = divmod(gs, NSUB)
                    c0 = s * SUB
                    zb_ps = z_ps.tile([128, SUB], f32, tag="zt_ps",
                                      name=f"zb_ps_{gs}")
                    nc.tensor.matmul(
                        zb_ps, whB_sb, th_t[sb][:, c0 : c0 + SUB],
                        start=True, stop=True,
                    )
                    nc.scalar.activation(
                        out=ez_t[sb][:, c0 : c0 + SUB], in_=zb_ps, func=AF.Exp
                    )
                    nc.vector.tensor_mul(
                        vt_t[sb][:, :, c0 : c0 + SUB],
                        vt_t[sb][:, :, c0 : c0 + SUB],
                        ez_t[sb][:, c0 : c0 + SUB]
                        .unsqueeze(1)
                        .broadcast_to([128, KC, SUB]),
                    )

                def emit_finalize(sb, ga=0, gb=GPS):
                    # finalize groups [ga, gb) of superblock sb (granular for
                    # the last superblock so the post-PE tail chain is short)
                    g0 = sb * GPS + ga
                    ng = gb - ga
                    lp = nc.allow_low_precision(
                        reason="fp16 group sums; fp32 internal accum"
                    )
                    lp.__enter__()
                    nc.vector.reduce_sum(
                        out=denom_sb[:, g0 : g0 + ng],
                        in_=ez_t[sb][0:1, ga * MSIZE : gb * MSIZE].rearrange(
                            "p (g n) -> p g n", n=MSIZE
                        ),
                        axis=AX.X,
                    )
                    tree = trp.tile([128, KC, ng, MSIZE // 2], f16, tag="tree",
                                    name=f"tree_{sb}_{ga}")
                    wv = vt_t[sb][:, :, ga * MSIZE : gb * MSIZE].rearrange(
                        "p c (g n) -> p c g n", n=MSIZE
                    )
                    nc.vector.tensor_add(
                        tree, wv[:, :, :, : MSIZE // 2], wv[:, :, :, MSIZE // 2 :]
                    )
                    nc.vector.tensor_add(
                        tree[:, :, :, : MSIZE // 4],
                        tree[:, :, :, : MSIZE // 4],
                        tree[:, :, :, MSIZE // 4 :],
                    )
                    nc.vector.tensor_add(
                        tree[:, :, :, : MSIZE // 8],
                        tree[:, :, :, : MSIZE // 8],
                        tree[:, :, :, MSIZE // 8 : MSIZE // 4],
                    )
                    nc.vector.reduce_sum(
                        out=cT_acc[:, :, g0 : g0 + ng],
                        in_=tree[:, :, :, : MSIZE // 8],
                        axis=AX.X,
                    )
                    lp.__exit__(None, None, None)
                    nc.sync.dma_start(
                        out=cT_d[:, g0 : g0 + ng].rearrange("(c p) n -> p c n", p=128),
                        in_=cT_acc[:, :, g0 : g0 + ng],
                    )

                # software-pipelined emission: mains(gs), content(gs-1),
                # score-chain(gs-2) -- every producer gets a full main block
                # to drain before its consumer issues
                # group-progress boundaries for granular last-superblock
                # finalize: after sub s the first (s+1)*SUB rows are weighted,
                # i.e. floor((s+1)*SUB/MSIZE) complete groups
                gcut = [min(GPS, (s + 1) * SUB // MSIZE) for s in range(NSUB)]
                for gs in range(NGS - 1):
                    emit_mains(gs)
                    if gs == 0:
                        emit_audio()
                    if gs >= 1:
                        emit_content(gs - 1)
                    if gs >= 2:
                        emit_score(gs - 2)
                        if (gs - 2) % NSUB == NSUB - 1:
                            emit_finalize((gs - 2) // NSUB)
                # final iteration: collapse the pipeline lag step by step so
                # the last score/finalize chains overlap the last mains, and
                # only one small granule remains after the PE finishes
                gs = NGS - 1
                emit_content(gs - 1)
                emit_score(gs - 2)
                emit_mains(gs)
                emit_finalize(NSB - 1, 0, gcut[0])
                emit_content(gs)
                emit_score(gs - 1)
                emit_finalize(NSB - 1, gcut[0], gcut[1])
                emit_score(gs)
                emit_finalize(NSB - 1, gcut[1], GPS)
                nc.sync.dma_start(out=denom_d, in_=denom_sb)

    nc.compile()
    return nc


def _prep_in_maps(inputs):
    audio = np.ascontiguousarray(np.asarray(inputs["audio"], np.float32))
    video = np.ascontiguousarray(np.asarray(inputs["video"], np.float32))
    def dev_chunks(w):  # [C*128, X] -> [128, C, X] (partition-major chunks)
        a = np.asarray(w)
        return np.ascontiguousarray(a.reshape(-1, 128, a.shape[-1]).transpose(1, 0, 2))

    WvideoT = dev_chunks(np.asarray(inputs["W_video"], np.float32).T.astype(np.float16))
    WaudioT = np.ascontiguousarray(np.asarray(inputs["W_audio"], np.float32).T.astype(np.float16))
    WgT = dev_chunks(np.asarray(inputs["W_g"], np.float32).T.astype(np.float16))
    WvT = dev_chunks(np.asarray(inputs["W_v"], np.float32).T.astype(np.float16))
    WhT = np.ascontiguousarray(np.asarray(inputs["W_h"], np.float32).T)
    b_video = np.ascontiguousarray(
        np.asarray(inputs["b_video"], np.float32).reshape(-1, 128).T
    )
    b_audio = np.ascontiguousarray(
        np.asarray(inputs["b_audio"], np.float32).reshape(-1, 128).T
    )

    a2 = audio.reshape(BT, ASIZE).astype(np.float16)
    v2 = video.reshape(BT, MSIZE, VSIZE).astype(np.float16)
    in_maps = []
    for c in range(NCORES):
        sl = slice(c * PER, (c + 1) * PER)
        vT = np.ascontiguousarray(v2[sl].reshape(R, VSIZE).T)
        audioT = np.ascontiguousarray(a2[sl].T)
        in_maps.append(
            {
                "vT": vT,
                "audioT": audioT,
                "WvideoT": WvideoT,
                "WaudioT": WaudioT,
                "WgT": WgT,
                "WvT": WvT,
                "WhT": WhT,
                "b_video": b_video,
                "b_audio": b_audio,
            }
        )
    return in_maps


def _run(inputs, trace=False, **spmd_kwargs):
    from concourse.bass_utils import run_bass_kernel_spmd

    if "nc" not in _cached:
        _cached["nc"] = _build_nc()
    nc = _cached["nc"]
    in_maps = _prep_in_maps(inputs)
    res = run_bass_kernel_spmd(
        nc, in_maps, core_ids=list(range(NCORES)), trace=trace, **spmd_kwargs
    )
    parts = [(r["cT"].astype(np.float32) / r["denom"].astype(np.float32)).T for r in res.results]
    out = np.concatenate(parts, axis=0).reshape(B, T, VSIZE)
    return np.ascontiguousarray(out.astype(np.float32)), res


def kernel(**inputs):
    out, _ = _run(inputs, trace=False)
    return out



# revision 38
# speedup vs baseline: 1.0111x; 1.0003x over previous
"""Trainium2 Bass kernel for the audio-visual attention model.

Math (per (b,t) sample, BT = 32*64 = 2048 of them):
    V   = video[b,t]                              # [48, 512]
    v   = relu(V @ W_video.T + b_video)           # [48, 512]
    a   = relu(audio[b,t] @ W_audio.T + b_audio)  # [512]
    inter   = a @ W_g.T                           # [48]
    content = v @ W_v.T + inter[:, None]          # [48, 48]
    z   = tanh(content) @ W_h.T                   # [48]
    alpha = softmax(z)
    out = alpha @ V                               # [512]

Strategy: data-parallel over BT across 8 cores (256 samples each, R = 256*48
= 12288 video rows per core).  The host pre-transposes the video shard to
V.T [512, 12288] (contraction dim on SBUF partitions), pre-arranges all
weights into their device layouts (so every weight DMA is contiguous per
partition), and converts the matmul chain to fp16 (values here are tiny, so
fp16's 10-bit mantissa gives ~7e-4 rel err while streaming the PE at
1 cycle/row -- float32 is 4x slower, float32r 2x).

Per-core device pipeline, over 8 superblocks of 1536 rows (3 sub-blocks of
512 rows each):
    vT.relu   = relu(W_video.T^T @ V.T + b_video)        PE + ACT  [h, rows]
    content.T = W_v.T^T @ vT.relu (+ ones^T @ inter)     PE        [48, rows]
    tanhc     = tanh(content.T)                          ACT
    zb        = (W_h.T replicated 128x)^T @ tanhc        PE        [128, rows]
                (z pre-broadcast to every partition -- no separate bcast)
    ezb       = exp(zb)   (no max-sub: |z| <= ~4)        ACT -> fp16 SBUF
    weighted  = V.T * ezb (in place, one 4-chunk op)     DVE 2x mode
    cT groups = halving-tree adds + small reduce         DVE (tree runs 2x)
    denom     = 48-group sums of ezb row 0               DVE
The audio phase (a.T, inter) is emitted after the first video sub-block so
the heavy matmul stream starts as soon as its DMAs land; it draws its PSUM
from the ct/z pools (idle until the first content step) so it never
contends with the video matmuls' tiles.  inter is flattened to a row-major
[1, 12288] single-partition row via two SBUF->SBUF DMAs on the gpsimd ring
(a DRAM roundtrip pays multi-microsecond HBM receipt latency under load).
Dummy matmul bursts keep the PE clock gate warm through the startup DMA
fill.  Outputs are the unnormalized c.T [512, 256] (fp16) plus denom
[1, 256]; the host divides and transposes.

Tail handling: the cT/denom output DMAs issue on the sync ring (the gpsimd
ring would head-of-line block behind their DVE-reduce waits), the last
superblock's finalize is emitted in 48-row-group granules interleaved with
the final score chain, and the closing iteration collapses the
content/score pipeline lag step by step -- together this cuts the post-PE
serial tail from ~19us to ~6us.  (Measured: the fp16 PE stream is the
roofline here -- 512-col matmuls sustain 259ns wall-to-wall regardless of
ldweights elision; fp8e4 DoubleRow measures 2.37x fp16 throughput, not the
cost model's 4x, so the 3-pass residual-compensated fp8 scheme that fits
the 2e-2 error gate is a net PE loss and was reverted.)
"""

import numpy as np

# Problem constants (hardcoded per harness contract).
B, T = 32, 64
ASIZE, VSIZE, HSIZE, MSIZE = 128, 512, 512, 48
NCORES = 8
BT = B * T                     # 2048
PER = BT // NCORES             # 256 samples per core
R = PER * MSIZE                # 12288 video rows per core
SUPER = 1536                   # rows per superblock (32 groups of 48)
NSB = R // SUPER               # 8 superblocks
SUB = 512                      # matmul moving-dim block
NSUB = SUPER // SUB            # 3
GPS = SUPER // MSIZE           # 32 groups per superblock

_cached = {}


def _build_nc():
    import concourse.bacc as bacc
    import concourse.mybir as mybir
    import concourse.tile as tile

    f32 = mybir.dt.float32
    f32r = mybir.dt.float32r
    f16 = mybir.dt.float16
    AF = mybir.ActivationFunctionType
    AX = mybir.AxisListType

    nc = bacc.Bacc(
        "TRN2",
        target_bir_lowering=False,
        debug=False,
        enable_asserts=False,
        num_devices=NCORES,
    )

    # ---- DRAM I/O ----
    vT_d = nc.dram_tensor("vT", [VSIZE, R], f16, kind="ExternalInput").ap()
    audioT_d = nc.dram_tensor("audioT", [ASIZE, PER], f16, kind="ExternalInput").ap()
    wvideoT_d = nc.dram_tensor("WvideoT", [128, VSIZE // 128, HSIZE], f16, kind="ExternalInput").ap()
    waudioT_d = nc.dram_tensor("WaudioT", [ASIZE, HSIZE], f16, kind="ExternalInput").ap()
    wgT_d = nc.dram_tensor("WgT", [128, HSIZE // 128, MSIZE], f16, kind="ExternalInput").ap()
    wvT_d = nc.dram_tensor("WvT", [128, HSIZE // 128, MSIZE], f16, kind="ExternalInput").ap()
    whT_d = nc.dram_tensor("WhT", [MSIZE, 1], f32, kind="ExternalInput").ap()
    bvideo_d = nc.dram_tensor("b_video", [128, HSIZE // 128], f32, kind="ExternalInput").ap()
    baudio_d = nc.dram_tensor("b_audio", [128, HSIZE // 128], f32, kind="ExternalInput").ap()
    cT_d = nc.dram_tensor("cT", [VSIZE, PER], f16, kind="ExternalOutput").ap()
    denom_d = nc.dram_tensor("denom", [1, PER], f16, kind="ExternalOutput").ap()

    KC = VSIZE // 128          # 4 contraction chunks for the main matmul
    HC = HSIZE // 128          # 4 h chunks

    with tile.TileContext(nc) as tc:
        with (
            tc.tile_pool(name="const", bufs=1) as const,
            tc.tile_pool(name="dram", bufs=1, space="DRAM") as dramp,
        ):
            # ---- constants / weights (host pre-arranged to device layout;
            # audio-path tensors first on the scalar ring: they gate the first
            # PE work) ----
            wvideoT_sb = const.tile([128, KC, HSIZE], f16)
            nc.scalar.dma_start(out=wvideoT_sb, in_=wvideoT_d)
            bvideo_sb = const.tile([128, HC], f32)
            nc.scalar.dma_start(out=bvideo_sb, in_=bvideo_d)
            audioT_sb = const.tile([128, PER], f16)
            nc.scalar.dma_start(out=audioT_sb, in_=audioT_d)
            waudioT_sb = const.tile([128, HSIZE], f16)
            nc.scalar.dma_start(out=waudioT_sb, in_=waudioT_d)
            baudio_sb = const.tile([128, HC], f32)
            nc.scalar.dma_start(out=baudio_sb, in_=baudio_d)
            wgT_sb = const.tile([128, HC, MSIZE], f16)
            nc.scalar.dma_start(out=wgT_sb, in_=wgT_d)
            whT_sb = const.tile([MSIZE, 1], f32)
            nc.scalar.dma_start(out=whT_sb, in_=whT_d)
            wvT_sb = const.tile([128, HC, MSIZE], f16)
            nc.scalar.dma_start(out=wvT_sb, in_=wvT_d)
            ones_m = const.tile([MSIZE, 128], f32)
            nc.vector.memset(ones_m, 1.0)
            whB_sb = const.tile([MSIZE, 128], f16)
            nc.scalar.mul(out=whB_sb, in_=ones_m, mul=whT_sb)
            # HAM warm-up: keep the PE busy during the initial DMA fill so the
            # clock gate is at 8/8 (2.4 GHz) before the real matmuls arrive
            warm_sb = const.tile([128, 64], f16)
            nc.vector.memset(warm_sb.bitcast(f32), 0.0)
            ones_f32 = const.tile([1, 128], f32)
            nc.vector.memset(ones_f32, 1.0)
            ones48 = const.tile([1, MSIZE], f16)
            nc.vector.tensor_copy(out=ones48, in_=ones_f32[:, :MSIZE])

            # persistent accumulators
            cT_acc = const.tile([128, KC, PER], f16)
            denom_sb = const.tile([1, PER], f16)
            interflat_all = const.tile([1, R], f16)

            # ---- pools (audio phase shares PSUM pools with the main loop so no
            # pool-release dependency delays the first video DMA) ----
            with (
                tc.tile_pool(name="vt", bufs=6) as vtp,
                tc.tile_pool(name="vrelu", bufs=4) as vrp,
                tc.tile_pool(name="tanhp", bufs=4) as thp,
                tc.tile_pool(name="small", bufs=3) as smp,
                tc.tile_pool(name="ezb", bufs=5) as ezp,
                tc.tile_pool(name="tree", bufs=2) as trp,
                tc.tile_pool(name="mm_ps", bufs=4, space="PSUM") as mm_ps,
                tc.tile_pool(name="ct_ps", bufs=2, space="PSUM") as ct_ps,
                tc.tile_pool(name="z_ps", bufs=2, space="PSUM") as z_ps,
            ):
                warm_ps = mm_ps.tile([64, 64], f32, tag="v_ps", name="warm_ps")

                def warm_burst(n):
                    for _ in range(n):
                        nc.tensor.matmul(
                            warm_ps, warm_sb[:, :64], warm_sb, start=True, stop=True
                        )

                warm_burst(100)

                def emit_audio():
                    # audio phase: a.T = relu(W_audio.T^T @ audio.T + b_audio)
                    # -- emitted after mains(0) so the heavy video stream
                    # starts as soon as its DMAs land.  PSUM comes from the
                    # ct/z pools (idle until gs>=1) so the audio matmuls
                    # don't contend with mains(0)'s v_ps tiles.
                    aT_sb = const.tile([128, HC, PER], f16)
                    for m in range(HC):
                        a_ps = z_ps.tile([128, PER], f32, tag="zt_ps",
                                         name=f"a_ps_{m}")
                        nc.tensor.matmul(
                            a_ps,
                            waudioT_sb[:, m * 128 : (m + 1) * 128],
                            audioT_sb,
                            start=True,
                            stop=True,
                        )
                        nc.scalar.activation(
                            out=aT_sb[:, m, :], in_=a_ps, func=AF.Relu,
                            bias=baudio_sb[:, m : m + 1],
                        )
                    # inter[bt, m] = a @ W_g.T, natural layout for a flat write
                    inter_sb = const.tile([128, PER // 128, MSIZE], f16)
                    for t in range(PER // 128):
                        i_ps = ct_ps.tile([128, MSIZE], f32, tag="c_ps",
                                          name=f"i_ps_{t}")
                        for k in range(HC):
                            nc.tensor.matmul(
                                i_ps,
                                aT_sb[:, k, t * 128 : (t + 1) * 128],
                                wgT_sb[:, k, :],
                                start=(k == 0),
                                stop=(k == HC - 1),
                            )
                        nc.scalar.copy(out=inter_sb[:, t, :], in_=i_ps)
                    # flatten inter [bt, m] row-major into a single-partition
                    # row via SBUF->SBUF DMA (no HBM roundtrip)
                    for t in range(PER // 128):
                        nc.gpsimd.dma_start(
                            out=interflat_all[:, t * 128 * MSIZE : (t + 1) * 128 * MSIZE],
                            in_=inter_sb[:, t, :],
                        )

                NGS = NSB * NSUB
                vt_t, vr_t, th_t, ez_t = {}, {}, {}, {}

                def emit_mains(gs):
                    sb, s = divmod(gs, NSUB)
                    if s == 0:
                        vt_t[sb] = vtp.tile([128, KC, SUPER], f16, tag="vt",
                                            name=f"vt_{sb}")
                        if sb == 0:
                            for ss in range(NSUB):
                                nc.sync.dma_start(
                                    out=vt_t[sb][:, :, ss * SUB : (ss + 1) * SUB],
                                    in_=vT_d[
                                        :, sb * SUPER + ss * SUB :
                                        sb * SUPER + (ss + 1) * SUB
                                    ].rearrange("(c p) n -> p c n", p=128),
                                )
                        else:
                            nc.sync.dma_start(
                                out=vt_t[sb],
                                in_=vT_d[:, sb * SUPER : (sb + 1) * SUPER].rearrange(
                                    "(c p) n -> p c n", p=128
                                ),
                            )
                        vr_t[sb] = vrp.tile([128, HC, SUPER], f16, tag="vrelu",
                                            name=f"vrelu_{sb}")
                        th_t[sb] = thp.tile([MSIZE, SUPER], f16, tag="tanhc",
                                            name=f"tanhc_{sb}")
                        ez_t[sb] = ezp.tile([128, SUPER], f16, tag="ezb",
                                            name=f"ezb_{sb}")
                    c0 = s * SUB
                    for m in range(HC):
                        v_ps = mm_ps.tile([128, SUB], f32, tag="v_ps",
                                          name=f"v_ps_{gs}_{m}")
                        for k in range(KC):
                            nc.tensor.matmul(
                                v_ps,
                                wvideoT_sb[:, k, m * 128 : (m + 1) * 128],
                                vt_t[sb][:, k, c0 : c0 + SUB],
                                start=(k == 0),
                                stop=(k == KC - 1),
                            )
                        nc.scalar.activation(
                            out=vr_t[sb][:, m, c0 : c0 + SUB], in_=v_ps,
                            func=AF.Relu, bias=bvideo_sb[:, m : m + 1],
                        )

                def emit_content(gs):
                    sb, s = divmod(gs, NSUB)
                    c0 = s * SUB
                    r0 = sb * SUPER
                    c_ps = ct_ps.tile([MSIZE, SUB], f32, tag="c_ps",
                                      name=f"c_ps_{gs}")
                    nc.tensor.matmul(
                        c_ps, ones48,
                        interflat_all[:, r0 + c0 : r0 + c0 + SUB],
                        start=True, stop=False,
                    )
                    for k in range(HC):
                        nc.tensor.matmul(
                            c_ps, wvT_sb[:, k, :], vr_t[sb][:, k, c0 : c0 + SUB],
                            start=False, stop=(k == HC - 1),
                        )
                    nc.scalar.activation(
                        out=th_t[sb][:, c0 : c0 + SUB], in_=c_ps, func=AF.Tanh
                    )

                def emit_score(gs):
                    sb, s = divmod(gs, NSUB)
                    c0 = s * SUB
                    zb_ps = z_ps.tile([128, SUB], f32, tag="zt_ps",
                                      name=f"zb_ps_{gs}")
                    nc.tensor.matmul(
                        zb_ps, whB_sb, th_t[sb][:, c0 : c0 + SUB],
                        start=True, stop=True,
                    )
                    nc.scalar.activation(
                        out=ez_t[sb][:, c0 : c0 + SUB], in_=zb_ps, func=AF.Exp
                    )
                    nc.vector.tensor_mul(
                        vt_t[sb][:, :, c0 : c0 + SUB],
                        vt_t[sb][:, :, c0 : c0 + SUB],
                        ez_t[sb][:, c0 : c0 + SUB]
                        .unsqueeze(1)
                        .broadcast_to([128, KC, SUB]),
                    )

                def emit_finalize(sb, ga=0, gb=GPS):
                    # finalize groups [ga, gb) of superblock sb (granular for
                    # the last superblock so the post-PE tail chain is short)
                    g0 = sb * GPS + ga
                    ng = gb - ga
                    lp = nc.allow_low_precision(
                        reason="fp16 group sums; fp32 internal accum"
                    )
                    lp.__enter__()
                    nc.vector.reduce_sum(
                        out=denom_sb[:, g0 : g0 + ng],
                        in_=ez_t[sb][0:1, ga * MSIZE : gb * MSIZE].rearrange(
                            "p (g n) -> p g n", n=MSIZE
                        ),
                        axis=AX.X,
                    )
                    tree = trp.tile([128, KC, ng, MSIZE // 2], f16, tag="tree",
                                    name=f"tree_{sb}_{ga}")
                    wv = vt_t[sb][:, :, ga * MSIZE : gb * MSIZE].rearrange(
                        "p c (g n) -> p c g n", n=MSIZE
                    )
                    nc.vector.tensor_add(
                        tree, wv[:, :, :, : MSIZE // 2], wv[:, :, :, MSIZE // 2 :]
                    )
                    nc.vector.tensor_add(
                        tree[:, :, :, : MSIZE // 4],
                        tree[:, :, :, : MSIZE // 4],
                        tree[:, :, :, MSIZE // 4 :],
                    )
                    nc.vector.tensor_add(
                        tree[:, :, :, : MSIZE // 8],
                        tree[:, :, :, : MSIZE // 8],
                        tree[:, :, :, MSIZE // 8 : MSIZE // 4],
                    )
                    nc.vector.reduce_sum(
                        out=cT_acc[:, :, g0 : g0 + ng],
                        in_=tree[:, :, :, : MSIZE // 8],
                        axis=AX.X,
                    )
                    lp.__exit__(None, None, None)
                    nc.sync.dma_start(
                        out=cT_d[:, g0 : g0 + ng].rearrange("(c p) n -> p c n", p=128),
                        in_=cT_acc[:, :, g0 : g0 + ng],
                    )

                # software-pipelined emission: mains(gs), content(gs-1),
                # score-chain(gs-2) -- every producer gets a full main block
                # to drain before its consumer issues
                # group-progress boundaries for granular last-superblock
                # finalize: after sub s the first (s+1)*SUB rows are weighted,
                # i.e. floor((s+1)*SUB/MSIZE) complete groups
                gcut = [min(GPS, (s + 1) * SUB // MSIZE) for s in range(NSUB)]
                for gs in range(NGS - 1):
                    emit_mains(gs)
                    if gs == 0:
                        emit_audio()
                    if gs >= 1:
                        emit_content(gs - 1)
                    if gs >= 2:
                        emit_score(gs - 2)
                        if (gs - 2) % NSUB == NSUB - 1:
                            emit_finalize((gs - 2) // NSUB)
                # final iteration: collapse the pipeline lag step by step so
                # the last score/finalize chains overlap the last mains, and
                # only one small granule remains after the PE finishes
                gs = NGS - 1
                emit_content(gs - 1)
                emit_score(gs - 2)
                emit_mains(gs)
                emit_finalize(NSB - 1, 0, gcut[0])
                emit_content(gs)
                emit_score(gs - 1)
                emit_finalize(NSB - 1, gcut[0], gcut[1])
                emit_score(gs)
                emit_finalize(NSB - 1, gcut[1], GPS)
                nc.sync.dma_start(out=denom_d, in_=denom_sb)

    nc.compile()
    return nc


def _prep_in_maps(inputs):
    audio = np.ascontiguousarray(np.asarray(inputs["audio"], np.float32))
    video = np.ascontiguousarray(np.asarray(inputs["video"], np.float32))
    def dev_chunks(w):  # [C*128, X] -> [128, C, X] (partition-major chunks)
        a = np.asarray(w)
        return np.ascontiguousarray(a.reshape(-1, 128, a.shape[-1]).transpose(1, 0, 2))

    WvideoT = dev_chunks(np.asarray(inputs["W_video"], np.float32).T.astype(np.float16))
    WaudioT = np.ascontiguousarray(np.asarray(inputs["W_audio"], np.float32).T.astype(np.float16))
    WgT = dev_chunks(np.asarray(inputs["W_g"], np.float32).T.astype(np.float16))
    WvT = dev_chunks(np.asarray(inputs["W_v"], np.float32).T.astype(np.float16))
    WhT = np.ascontiguousarray(np.asarray(inputs["W_h"], np.float32).T)
    b_video = np.ascontiguousarray(
        np.asarray(inputs["b_video"], np.float32).reshape(-1, 128).T
    )
    b_audio = np.ascontiguousarray(
        np.asarray(inputs["b_audio"], np.float32).reshape(-1, 128).T
    )

    a2 = audio.reshape(BT, ASIZE).astype(np.float16)
    v2 = video.reshape(BT, MSIZE, VSIZE).astype(np.float16)
    in_maps = []
    for c in range(NCORES):
        sl = slice(c * PER, (c + 1) * PER)
        vT = np.ascontiguousarray(v2[sl].reshape(R, VSIZE).T)
        audioT = np.ascontiguousarray(a2[sl].T)
        in_maps.append(
            {
                "vT": vT,
                "audioT": audioT,
                "WvideoT": WvideoT,
                "WaudioT": WaudioT,
                "WgT": WgT,
                "WvT": WvT,
                "WhT": WhT,
                "b_video": b_video,
                "b_audio": b_audio,
            }
        )
    return in_maps


def _run(inputs, trace=False, **spmd_kwargs):
    from concourse.bass_utils import run_bass_kernel_spmd

    if "nc" not in _cached:
        _cached["nc"] = _build_nc()
    nc = _cached["nc"]
    in_maps = _prep_in_maps(inputs)
    res = run_bass_kernel_spmd(
        nc, in_maps, core_ids=list(range(NCORES)), trace=trace, **spmd_kwargs
    )
    parts = [(r["cT"].astype(np.float32) / r["denom"].astype(np.float32)).T for r in res.results]
    out = np.concatenate(parts, axis=0).reshape(B, T, VSIZE)
    return np.ascontiguousarray(out.astype(np.float32)), res


def kernel(**inputs):
    out, _ = _run(inputs, trace=False)
    return out

